# revision 42
# baseline (speedup 1.0000x reference)
"""TRN2 Bass kernel for nn_CLIPVisionTower attention block.

Data-parallel over batch: 16 images across 8 NeuronCores (2 each).
Per core, for each local image b:
  hsT = hs^T                                   (PE transposes)
  qT = wq_s @ hsT + bq_s ; kT = wk @ hsT + bk  ([E, T] layout, fp32r)
  v  = hs @ wv^T                               ([T, E] layout; bias folded into bo_eff)
  score += q @ k^T over full E                 (PSUM accumulate over local images)
  mean_keys = Sel @ kT (head-mean)             (+ PE transpose to [T, D])
  per head h: logitsT[s,t] = kh^T q ; expT = exp(logitsT + logsize[s]) (ACT bias trick)
              [num; denom] = [vh | 1]^T @ expT (PSUM accumulate over s-chunks)
  normalize with reciprocal + PE broadcast matmul; attn = outT^T @ wo^T + bo_eff
All matmuls run in fp32r (fp32 rounded to 12-bit mantissa; full PE rate at N>=256).
att_score partial sums are reduced across cores on the host.
"""

import numpy as np

import concourse.mybir as mybir
import concourse.tile as tile
from concourse import bacc
from concourse.bass_utils import run_bass_kernel_spmd

dt = mybir.dt
AF = mybir.ActivationFunctionType
ALU = mybir.AluOpType

B, T, E, H, D = 16, 577, 1024, 16, 64
NCORES = 8
NB = B // NCORES          # local batch = 2
TP = 578                  # padded T (even; runt chunk 66)
SC = 5                    # s/t chunks of 128 (last = 66)
CH = [(0, 128), (128, 128), (256, 128), (384, 128), (512, 66)]   # (start, len) t/s chunks
NCH = [(0, 512), (512, 66)]                                      # matmul N chunks (bank-aligned)
EC = 8                    # e chunks of 128
SCALE = D ** -0.5


def _round_fp32r(x):
    b = np.ascontiguousarray(x, np.float32).view(np.uint32).astype(np.int64)
    b = (b + 0x800) & 0xFFFFF000
    return (b & 0xFFFFFFFF).astype(np.uint32).view(np.float32)


_CACHE = {}


def _build():
    if "nc" in _CACHE:
        return _CACHE["nc"]

    nc = bacc.Bacc("TRN2", target_bir_lowering=False, debug=False)

    hsx = nc.dram_tensor("hsx", [NB, T, E], dt.float32r, kind="ExternalInput")
    wqt = nc.dram_tensor("wqt", [E, E], dt.float32r, kind="ExternalInput")
    wkt = nc.dram_tensor("wkt", [E, E], dt.float32r, kind="ExternalInput")
    wvt = nc.dram_tensor("wvt", [E, E], dt.float32r, kind="ExternalInput")
    wot = nc.dram_tensor("wot", [E, E], dt.float32r, kind="ExternalInput")
    bq8 = nc.dram_tensor("bq8", [EC, 128], dt.float32, kind="ExternalInput")
    bk8 = nc.dram_tensor("bk8", [EC, 128], dt.float32, kind="ExternalInput")
    boe = nc.dram_tensor("boe", [E], dt.float32, kind="ExternalInput")
    lsr = nc.dram_tensor("lsr", [NB, SC, 128], dt.float32, kind="ExternalInput")
    iden = nc.dram_tensor("iden", [128, 128], dt.float32r, kind="ExternalInput")
    seli = nc.dram_tensor("seli", [128, 64], dt.float32r, kind="ExternalInput")

    attn = nc.dram_tensor("attn", [NB, T, E], dt.float32, kind="ExternalOutput")
    meank = nc.dram_tensor("meank", [NB, T, D], dt.float32, kind="ExternalOutput")
    score = nc.dram_tensor("score", [T, T], dt.float32, kind="ExternalOutput")

    with tile.TileContext(nc) as tc:
        with (
            tc.tile_pool(name="const", bufs=1) as cpool,
            tc.tile_pool(name="data", bufs=2) as sb,
            tc.tile_pool(name="pslg", bufs=2, space="PSUM") as pslg,
            tc.tile_pool(name="psnum", bufs=2, space="PSUM") as psnum,
        ):
            ident = cpool.tile([128, 128], dt.float32r)
            nc.sync.dma_start(ident[:], iden[:])
            qT = [sb.tile([128, EC, TP], dt.float32r, tag="qT", name=f"qT{b}") for b in range(NB)]
            kT = [sb.tile([128, EC, TP], dt.float32r, tag="kT", name=f"kT{b}") for b in range(NB)]
            vaug = [sb.tile([128, SC, H, 66], dt.float32r, tag="vaug", name=f"vaug{b}") for b in range(NB)]

            # ---------- P0 + P1: transpose hidden states, projections ----------
            with (
                tc.tile_pool(name="p01", bufs=2) as p01,
                tc.tile_pool(name="w", bufs=2) as wpool,
            ):
                hsT = [p01.tile([128, EC, TP], dt.float32r, tag="hsT", name=f"hsT{b}") for b in range(NB)]
                HALF = EC // 2
                wq_halves = []

                def emit_p0(b, dmaq=None):
                    for u, (t0, tl) in enumerate(CH):
                        dl = min(tl, T - t0)
                        for half in range(2):
                            hst = p01.tile([128, 512], dt.float32r, tag="hsin", bufs=6)
                            if dl < 128:
                                nc.vector.memset(hst[:].bitcast(dt.float32), 0.0)
                            nc.sync.dma_start(
                                hst[:dl, :], hsx[b, t0 : t0 + dl, half * 512 : half * 512 + 512]
                            )
                            if dmaq:
                                dmaq.pop(0)()
                            for c4 in range(4):
                                c = half * 4 + c4
                                tp = pslg.tile([128, TP], dt.float32, tag="lg")
                                nc.tensor.transpose(
                                    tp[:, :tl].bitcast(dt.float32r),
                                    hst[:tl, c4 * 128 : (c4 + 1) * 128],
                                    ident[:tl, :tl],
                                )
                                if c % 2 == 0:
                                    nc.vector.tensor_copy(hsT[b][:, c, t0 : t0 + tl], tp[:, :tl].bitcast(dt.float32r))
                                else:
                                    nc.scalar.copy(hsT[b][:, c, t0 : t0 + tl], tp[:, :tl].bitcast(dt.float32r))

                # interleave wq chunk DMAs into batch-0 transposes so q-proj starts unpaced
                for half in range(2):
                    wq_halves.append(wpool.tile([128, HALF, E], dt.float32r, tag="w", name=f"wq{half}"))

                def _wq_dma(half, c):
                    cc = half * HALF + c
                    return lambda: nc.sync.dma_start(
                        wq_halves[half][:, c, :], wqt[cc * 128 : (cc + 1) * 128, :]
                    )

                dmaq = [_wq_dma(h2, c) for h2 in range(2) for c in range(HALF)]
                dmaq += [lambda: None, lambda: None]
                emit_p0(0, dmaq)
                sel = cpool.tile([128, 64], dt.float32r)
                nc.sync.dma_start(sel[:], seli[:])
                ls = cpool.tile([128, NB, SC], dt.float32)
                nc.sync.dma_start(ls[:], lsr.rearrange("b c p -> p b c"))
                bq = cpool.tile([128, EC], dt.float32)
                nc.sync.dma_start(bq[:], bq8.rearrange("c p -> p c"))
                bk = cpool.tile([128, EC], dt.float32)
                nc.sync.dma_start(bk[:], bk8.rearrange("c p -> p c"))
                bo_b = cpool.tile([128, E], dt.float32)
                nc.sync.dma_start(bo_b[:], boe[None, :].to_broadcast((128, E)))
                emit_p0(1)

                # q/k projections; weights streamed as 16KB half-tiles
                for (wt, dest, bias) in ((wqt, qT, bq), (wkt, kT, bk)):
                    if wt is wqt:
                        wh = wq_halves
                    else:
                        wh = []
                        for half in range(2):
                            whs = wpool.tile([128, HALF, E], dt.float32r, tag="w", name=f"wk{half}")
                            for c in range(HALF):
                                cc = half * HALF + c
                                nc.sync.dma_start(whs[:, c, :], wt[cc * 128 : (cc + 1) * 128, :])
                            wh.append(whs)
                    for b in range(NB):
                        for m in range(EC):
                            acc = psnum.tile([128, E], dt.float32, tag="num")
                            for k8 in range(EC):
                                for n0, nl in NCH:
                                    nc.tensor.matmul(
                                        acc[:, n0 : n0 + nl],
                                        wh[k8 // HALF][:, k8 % HALF, m * 128 : (m + 1) * 128],
                                        hsT[b][:, k8, n0 : n0 + nl],
                                        start=(k8 == 0),
                                        stop=(k8 == EC - 1),
                                    )
                            nc.scalar.add(dest[b][:, m, :], acc[:, :TP], bias[:, m : m + 1])
                # v projection: [T, E] layout, no bias
                wh = []
                for half in range(2):
                    whs = wpool.tile([128, HALF, E], dt.float32r, tag="w", name=f"wv{half}")
                    for c in range(HALF):
                        cc = half * HALF + c
                        for q2 in range(2):
                            nc.sync.dma_start(
                                whs[:, c, q2 * 512 : (q2 + 1) * 512],
                                wvt[cc * 128 : (cc + 1) * 128, q2 * 512 : (q2 + 1) * 512],
                            )
                    wh.append(whs)
                for b in range(NB):
                    for u, (t0, tl) in enumerate(CH):
                        acc = psnum.tile([128, E], dt.float32, tag="num")
                        for k8 in range(EC):
                            for n in range(2):
                                nc.tensor.matmul(
                                    acc[:tl, n * 512 : (n + 1) * 512],
                                    hsT[b][:, k8, t0 : t0 + tl],
                                    wh[k8 // HALF][:, k8 % HALF, n * 512 : (n + 1) * 512],
                                    start=(k8 == 0),
                                    stop=(k8 == EC - 1),
                                )
                        if tl < 128:
                            nc.vector.memset(vaug[b][64:128, u, :, 0:64].bitcast(dt.float32), 0.0)
                        nc.scalar.copy(
                            vaug[b][:tl, u, :, 0:64],
                            acc[:tl].rearrange("p (h d) -> p h d", d=64),
                        )
                        nc.vector.memset(vaug[b][:, u, :, 64:66].bitcast(dt.float32), 1.0)

            # wo loads during P2
            with tc.tile_pool(name="wo", bufs=1) as wopool:
                wo_sb = wopool.tile([128, EC, E], dt.float32r, tag="wo")
                for c in range(EC):
                    nc.sync.dma_start(wo_sb[:, c, :], wot[c * 128 : (c + 1) * 128, :])

                # ---------- P2: att_score + mean_keys ----------
                with (
                    tc.tile_pool(name="p2", bufs=2) as p2,
                    tc.tile_pool(name="p3", bufs=3) as p3,
                    tc.tile_pool(name="p3d", bufs=4) as p3d,
                    tc.tile_pool(name="p3s", bufs=1) as p3s,
                ):
                    # mean_keys head-sum matmuls + row copies (transposes deferred)
                    mks_l = []
                    for b in range(NB):
                        acc = pslg.tile([64, TP], dt.float32, tag="lg")
                        for k8 in range(EC):
                            for n0, nl in NCH:
                                nc.tensor.matmul(
                                    acc[:, n0 : n0 + nl],
                                    sel[:],
                                    kT[b][:, k8, n0 : n0 + nl],
                                    start=(k8 == 0),
                                    stop=(k8 == EC - 1),
                                )
                        mks = p2.tile([64, TP], dt.float32, tag="mks", bufs=2, name=f"mks{b}")
                        nc.scalar.copy(mks[:], acc[:])
                        mks_l.append(mks)

                    def emit_score_chunk(t5):
                        t0, tl = CH[t5]
                        dl = min(tl, T - t0)
                        acc = psnum.tile([128, E], dt.float32, tag="num", name=f"sc{t5}")
                        for b in range(NB):
                            for k8 in range(EC):
                                for n0, nl in NCH:
                                    nc.tensor.matmul(
                                        acc[:tl, n0 : n0 + nl],
                                        qT[b][:, k8, t0 : t0 + tl],
                                        kT[b][:, k8, n0 : n0 + nl],
                                        start=(b == 0 and k8 == 0),
                                        stop=(b == NB - 1 and k8 == EC - 1),
                                    )
                        ssb = p2.tile([128, TP], dt.float32, tag="ssb", bufs=2)
                        nc.vector.tensor_scalar_mul(ssb[:tl], acc[:tl, :TP], 1.0 / (B * H))
                        qd = dl // 2
                        for q2 in range(2):
                            r0 = q2 * qd
                            r1 = dl if q2 == 1 else (q2 + 1) * qd
                            nc.sync.dma_start(score[t0 + r0 : t0 + r1, :], ssb[r0:r1, :T])

                    def emit_meank_tail(b, t5):
                        t0, tl = CH[t5]
                        dl = min(tl, T - t0)
                        tp = pslg.tile([128, TP], dt.float32, tag="lg")
                        nc.tensor.transpose(tp[:tl, :64], mks_l[b][:, t0 : t0 + tl], ident[:64, :64].bitcast(dt.float32))
                        mkt = p2.tile([128, 64], dt.float32, tag="mkt")
                        nc.scalar.copy(mkt[:tl, :], tp[:tl, :64])
                        nc.sync.dma_start(meank[b, t0 : t0 + dl, :], mkt[:dl, :])

                    # interleave plan: before head (b, h) emit at most one deferred P2 item
                    p2q = [("s", 0), ("s", 1), ("s", 2), ("s", 3), ("s", 4)] + [
                        ("m", b * SC + t5) for b in range(NB) for t5 in range(SC)
                    ]

                    for b in range(NB):
                        outT = p3s.tile([128, EC, TP], dt.float32r, tag="outT", name=f"outT{b}")
                        pend = []
                        nums = {}

                        def emit_norm_of_pend():
                            for (hq, dq) in pend:
                                cq, oq = hq // 2, (hq % 2) * 64
                                rbt = p3d.tile([128, TP], dt.float32r, tag="rbt", bufs=2)
                                nc.gpsimd.partition_broadcast(rbt[:, :], dq[0:1, :], channels=128)
                                nc.vector.tensor_tensor(
                                    outT[oq : oq + 64, cq, :], outT[oq : oq + 64, cq, :], rbt[oq : oq + 64, :], ALU.mult
                                )

                        def emit_av(h, uu, exp_):
                            nonlocal pend
                            c, o = h // 2, (h % 2) * 64
                            if uu == 0:
                                nums[h] = psnum.tile([128, E], dt.float32, tag="num", name=f"num{b}_{h}")
                            num = nums[h]
                            for n0, nl in NCH:
                                nc.tensor.matmul(
                                    num[:66, n0 : n0 + nl],
                                    vaug[b][:, uu, h, :],
                                    exp_[:, n0 : n0 + nl],
                                    start=(uu == 0),
                                    stop=(uu == SC - 1),
                                )
                            if uu == SC - 1:
                                dtmp = p3d.tile([66, TP], dt.float32r, tag="dtmp", bufs=2)
                                with nc.allow_low_precision(reason="softmax denom reciprocal"):
                                    nc.vector.reciprocal(dtmp[0:2, :], num[64:66, :TP])
                                nc.vector.tensor_copy(outT[o : o + 64, c, :], num[0:64, :TP])
                                del nums[h]
                                emit_norm_of_pend()
                                pend = [(h, dtmp)]

                        exq = []
                        for h in range(H):
                            if p2q and h % 2 == 0:
                                kind, idx = p2q.pop(0)
                                if kind == "s":
                                    emit_score_chunk(idx)
                                else:
                                    emit_meank_tail(idx // SC, idx % SC)
                            c, o = h // 2, (h % 2) * 64
                            for u, (s0, sl) in enumerate(CH):
                                lg = pslg.tile([128, TP], dt.float32, tag="lg", name=f"lg{b}_{h}_{u}")
                                for n0, nl in NCH:
                                    nc.tensor.matmul(
                                        lg[:sl, n0 : n0 + nl],
                                        kT[b][o : o + 64, c, s0 : s0 + sl],
                                        qT[b][o : o + 64, c, n0 : n0 + nl],
                                        start=True,
                                        stop=True,
                                    )
                                ex = p3.tile([128, TP], dt.float32r, tag="ex", bufs=5)
                                nc.scalar.activation(ex[:], lg[:, :TP], AF.Exp, bias=ls[:, b, u : u + 1])
                                exq.append((h, u, ex))
                                if len(exq) >= 4:
                                    hh, uu, exp_ = exq.pop(0)
                                    emit_av(hh, uu, exp_)
                        for hh, uu, exp_ in exq:
                            emit_av(hh, uu, exp_)
                        exq = []
                        emit_norm_of_pend()
                        pend = []
                        # output projection
                        for t5, (t0, tl) in enumerate(CH):
                            dl = min(tl, T - t0)
                            acc = psnum.tile([128, E], dt.float32, tag="num")
                            for c in range(EC):
                                for n in range(2):
                                    nc.tensor.matmul(
                                        acc[:tl, n * 512 : (n + 1) * 512],
                                        outT[:, c, t0 : t0 + tl],
                                        wo_sb[:, c, n * 512 : (n + 1) * 512],
                                        start=(c == 0),
                                        stop=(c == EC - 1),
                                    )
                            asb = p3.tile([128, E], dt.float32, tag="asb", bufs=2)
                            nc.vector.tensor_tensor(asb[:dl, :], acc[:dl, :], bo_b[:dl, :], ALU.add)
                            for q2 in range(2):
                                nc.sync.dma_start(
                                    attn[b, t0 : t0 + dl, q2 * 512 : (q2 + 1) * 512],
                                    asb[:dl, q2 * 512 : (q2 + 1) * 512],
                                )

                    while p2q:
                        kind, idx = p2q.pop(0)
                        if kind == "s":
                            emit_score_chunk(idx)
                        else:
                            emit_meank_tail(idx // SC, idx % SC)

    nc.compile()
    _CACHE["nc"] = nc
    return nc


def _prep_inputs(hidden_states, size, wq, bq, wk, bk, wv, bv, wo, bo):
    hs = np.ascontiguousarray(hidden_states, np.float32)
    size = np.asarray(size, np.float32)
    wq = np.asarray(wq, np.float32)
    bq = np.asarray(bq, np.float32)
    wk = np.asarray(wk, np.float32)
    bk = np.asarray(bk, np.float32)
    wv = np.asarray(wv, np.float32)
    bv = np.asarray(bv, np.float32)
    wo = np.asarray(wo, np.float32)
    bo = np.asarray(bo, np.float32)

    wqt = _round_fp32r(np.ascontiguousarray((wq * SCALE).T))
    wkt = _round_fp32r(np.ascontiguousarray(wk.T))
    wvt = _round_fp32r(np.ascontiguousarray(wv.T))
    wot = _round_fp32r(np.ascontiguousarray(wo.T))
    bq8 = np.ascontiguousarray((bq * SCALE).reshape(EC, 128))
    bk8 = np.ascontiguousarray(bk.reshape(EC, 128))
    boe = np.ascontiguousarray(bo + wo @ bv)

    logsize = np.log(size).astype(np.float32)  # [B, T]
    flat = np.full((B, SC * 128), np.float32(-1e30), np.float32)
    flat[:, :T] = logsize
    lsr = flat.reshape(B, SC, 128)

    iden = np.eye(128, dtype=np.float32)
    seli = _round_fp32r(np.tile(np.eye(64, dtype=np.float32) / H, (2, 1)))

    shared = {
        "wqt": wqt, "wkt": wkt, "wvt": wvt, "wot": wot,
        "bq8": bq8, "bk8": bk8, "boe": boe,
        "iden": iden, "seli": seli,
    }
    in_maps = []
    for core in range(NCORES):
        b0 = core * NB
        m = dict(shared)
        m["hsx"] = _round_fp32r(hs[b0 : b0 + NB])
        m["lsr"] = np.ascontiguousarray(lsr[b0 : b0 + NB])
        in_maps.append(m)
    return in_maps


def kernel(hidden_states, size, wq, bq, wk, bk, wv, bv, wo, bo, _trace=False, **_kw):
    nc = _build()
    in_maps = _prep_inputs(hidden_states, size, wq, bq, wk, bk, wv, bv, wo, bo)
    res = run_bass_kernel_spmd(nc, in_maps, list(range(NCORES)), trace=_trace)
    attn = np.concatenate([r["attn"] for r in res.results], axis=0)
    meank = np.concatenate([r["meank"] for r in res.results], axis=0)
    score = np.sum([r["score"] for r in res.results], axis=0, dtype=np.float32)
    if _trace:
        kernel._last_results = res
    return attn, meank, score
